# revision 8
# baseline (speedup 1.0000x reference)
"""Batch-softmax attention kernel for 8 trn2 NeuronCores.

Math (reference):
  q = x @ Wq^T + bq ; k = x @ Wk^T + bk ; v = x @ Wv^T + bv     (B,N,256)
  logits[b,n,m] = q[b,n,:] . k[b,m,:]
  attn = softmax over the BATCH axis b (32 entries per (n,m) site)
  out = attn @ v

Distribution: data-parallel over N (sequence). Core c owns n-shard
[c*NSH,(c+1)*NSH). Each core recomputes k for all m locally (no
collectives), streams x over m-tiles, and uses
  out = (P @ [x|1]) @ [Wv^T;bv]        (P = attn)
to skip the v projection entirely (the ones column yields rowsum(P),
which carries the bv term through the second matmul).

Layouts: attention tiles are (m-partition, n-free) i.e. logits^T, so the
batch softmax is elementwise across per-b tiles and P^T directly feeds
P@x as the stationary operand. exp needs no max-subtraction: |logits| is
~<=40, safe in fp32. E is bf16 (range), matmul operands fp16, all
accumulation fp32. Biases fold into the projections via an augmented
contraction row (bias_row x ones_row).
"""

import numpy as np
from contextlib import ExitStack

import concourse.bass as bass
import concourse.tile as tile
from concourse import mybir, bacc

f32 = mybir.dt.float32
f16 = mybir.dt.float16
bf16 = mybir.dt.bfloat16
i32 = mybir.dt.int32
ADD = mybir.AluOpType.add
MUL = mybir.AluOpType.mult
EQ = mybir.AluOpType.is_equal
EXP = mybir.ActivationFunctionType.Exp
COPY = mybir.ActivationFunctionType.Copy

N_CORES = 8


def build_graph(nc, B, N, D, O, NSH, CH=2, BG=4, BQ=4):
    MT = N // 128          # m-tiles
    NCH = MT // CH         # chunks
    NG = B // BG           # logits-psum b-groups
    DT, OT = D // 128, O // 128
    M = CH * 128           # m positions per chunk

    xf = nc.dram_tensor("xf", [B, N, D], f32, kind="ExternalInput")
    xs = nc.dram_tensor("xs", [B, NSH, D], f32, kind="ExternalInput")
    wq = nc.dram_tensor("wq", [O, D], f32, kind="ExternalInput")
    wk = nc.dram_tensor("wk", [O, D], f32, kind="ExternalInput")
    wv = nc.dram_tensor("wv", [O, D], f32, kind="ExternalInput")
    bq = nc.dram_tensor("bq", [O], f32, kind="ExternalInput")
    bk = nc.dram_tensor("bk", [O], f32, kind="ExternalInput")
    bv = nc.dram_tensor("bv", [O], f32, kind="ExternalInput")
    out = nc.dram_tensor("out", [B, NSH, O], f32, kind="ExternalOutput")

    with ExitStack() as ctx:
        tc = ctx.enter_context(tile.TileContext(nc))
        const = ctx.enter_context(tc.tile_pool(name="const", bufs=1))
        wtmp = ctx.enter_context(tc.tile_pool(name="wtmp", bufs=2))
        xfp = ctx.enter_context(tc.tile_pool(name="xfp", bufs=3))
        xtp = ctx.enter_context(tc.tile_pool(name="xtp", bufs=2))
        ktp = ctx.enter_context(tc.tile_pool(name="ktp", bufs=4))
        ep = ctx.enter_context(tc.tile_pool(name="ep", bufs=CH * NG + 4))
        pp = ctx.enter_context(tc.tile_pool(name="pp", bufs=CH * NG + 2))
        zp = ctx.enter_context(tc.tile_pool(name="zp", bufs=10))
        zp2 = ctx.enter_context(tc.tile_pool(name="zp2", bufs=3))
        outp = ctx.enter_context(tc.tile_pool(name="outp", bufs=3))
        ps_l = ctx.enter_context(tc.tile_pool(name="ps_l", bufs=3, space="PSUM"))
        ps_t = ctx.enter_context(tc.tile_pool(name="ps_t", bufs=3, space="PSUM"))
        ps_m = ctx.enter_context(tc.tile_pool(name="ps_m", bufs=2, space="PSUM"))

        # ---- constants
        ia = const.tile([128, 128], i32)
        ib = const.tile([128, 128], i32)
        nc.gpsimd.iota(ia[:], pattern=[[1, 128]], channel_multiplier=0)
        nc.gpsimd.iota(ib[:], pattern=[[0, 128]], channel_multiplier=1)
        ident = const.tile([128, 128], f16)
        nc.vector.tensor_tensor(ident[:], ia[:], ib[:], op=EQ)
        ones_row = const.tile([1, 512], f16)
        nc.vector.memset(ones_row[:], 1.0)

        # ---- weights: W^T (d-part, o-free) f16; bias rows (1,O) f16
        def load_wT(wdram, name):
            wT = [const.tile([128, O], f16, name=f"{name}T{d}", tag=f"{name}T{d}") for d in range(DT)]
            for ot in range(OT):
                wfull = wtmp.tile([128, D], f32, tag="wf")
                nc.sync.dma_start(wfull[:], wdram.ap()[ot * 128:(ot + 1) * 128, :])
                wh = wtmp.tile([128, D], f16, tag="wh")
                nc.vector.tensor_copy(wh[:], wfull[:])
                for d in range(DT):
                    ps = ps_m.tile([128, 128], f16, tag="mm")
                    nc.tensor.transpose(ps[:], wh[:, d * 128:(d + 1) * 128], ident[:])
                    nc.scalar.activation(wT[d][:, ot * 128:(ot + 1) * 128], ps[:], COPY)
            return wT

        def load_brow(bdram, name):
            bfl = wtmp.tile([1, O], f32, tag="bf")
            nc.sync.dma_start(bfl[:], bdram.ap()[:])
            bh = const.tile([1, O], f16, tag=name)
            nc.vector.tensor_copy(bh[:], bfl[:])
            return bh

        wqT = load_wT(wq, "wq")
        wkT = load_wT(wk, "wk")
        wvT = load_wT(wv, "wv")
        bq_r = load_brow(bq, "bq")
        bk_r = load_brow(bk, "bk")
        bv_r = load_brow(bv, "bv")

        # ---- setup: q^T (o-part, n-free) per b. slab slice (b,ot) at
        # [(b*OT+ot)*NSH].
        qT = const.tile([128, B * OT * NSH], f16)
        xss = const.tile([NSH, B * D], f16, tag="xss")
        xs3 = xss[:].rearrange("p (b d) -> p b d", b=B)
        nc.gpsimd.dma_start(xs3[:, :, :], xs.ap()[:, :, :].rearrange("b n d -> n b d"))
        for b in range(B):
            pst = ps_t.tile([128, DT * NSH], f16, tag="tp")
            for d in range(DT):
                nc.tensor.transpose(pst[:, d * NSH:(d + 1) * NSH],
                                    xs3[:, b, d * 128:(d + 1) * 128],
                                    ident[:NSH, :NSH])
            xsT = wtmp.tile([128, DT * NSH], f16, tag="xsT")
            nc.vector.tensor_copy(xsT[:], pst[:])
            psq = ps_m.tile([128, OT * NSH], f32, tag="mm")
            for ot in range(OT):
                sl = psq[:, ot * NSH:(ot + 1) * NSH]
                for d in range(DT):
                    nc.tensor.matmul(sl, wqT[d][:, ot * 128:(ot + 1) * 128],
                                     xsT[:, d * NSH:(d + 1) * NSH],
                                     start=(d == 0), stop=False)
                nc.tensor.matmul(sl, bq_r[:, ot * 128:(ot + 1) * 128],
                                 ones_row[:, :NSH], start=False, stop=True)
            nc.scalar.activation(qT[:, b * OT * NSH:(b + 1) * OT * NSH], psq[:], COPY)

        # ---- Y accumulator (n-part, b x (O+1)) f32
        YW = O + 1
        Y = const.tile([NSH, B * YW], f32)

        # ---- stream over chunks of CH m-tiles
        for ch in range(NCH):
            slabs = []
            for ci in range(CH):
                mt = ch * CH + ci
                slab = xfp.tile([128, B * (D + 1)], f16, tag="xslab")
                s3 = slab[:].rearrange("p (b d) -> p b d", b=B)
                nc.vector.memset(s3[:, :, D:D + 1], 1.0)
                nc.gpsimd.dma_start(
                    s3[:, :, 0:D],
                    xf.ap()[:, mt * 128:(mt + 1) * 128, :].rearrange("b n d -> n b d"))
                slabs.append(s3)

            Lps, Es = {}, {}
            for b0 in range(0, B, BQ):
                tps = [ps_t.tile([128, BQ * M], f16, name=f"tps{d}", tag="tp") for d in range(DT)]
                xT = [xtp.tile([128, BQ * M], f16, name=f"xT{d}", tag=f"xT{d}") for d in range(DT)]
                for bb in range(BQ):
                    for ci in range(CH):
                        for d in range(DT):
                            nc.tensor.transpose(
                                tps[d][:, (bb * CH + ci) * 128:(bb * CH + ci + 1) * 128],
                                slabs[ci][:, b0 + bb, d * 128:(d + 1) * 128], ident[:])
                for d in range(DT):
                    nc.vector.tensor_copy(xT[d][:], tps[d][:])
                for bb in range(BQ):
                    b = b0 + bb
                    psk = ps_m.tile([128, OT * M], f32, tag="mm")
                    for ot in range(OT):
                        sl = psk[:, ot * M:(ot + 1) * M]
                        for d in range(DT):
                            nc.tensor.matmul(sl, wkT[d][:, ot * 128:(ot + 1) * 128],
                                             xT[d][:, bb * M:(bb + 1) * M],
                                             start=(d == 0), stop=False)
                        nc.tensor.matmul(sl, bk_r[:, ot * 128:(ot + 1) * 128],
                                         ones_row[:, :M], start=False, stop=True)
                    kT = ktp.tile([128, OT * M], f16, tag="kT")
                    nc.scalar.activation(kT[:], psk[:], COPY)

                    g, j = b // BG, b % BG
                    for ci in range(CH):
                        if j == 0:
                            Lps[(ci, g)] = ps_l.tile([128, BG * NSH], f32, name="lps", tag="lps")
                        sl = Lps[(ci, g)][:, j * NSH:(j + 1) * NSH]
                        for ot in range(OT):
                            nc.tensor.matmul(
                                sl, kT[:, ot * M + ci * 128:ot * M + (ci + 1) * 128],
                                qT[:, (b * OT + ot) * NSH:(b * OT + ot + 1) * NSH],
                                start=(ot == 0), stop=(ot == OT - 1))
                        if j == BG - 1:
                            E = ep.tile([128, BG * NSH], bf16, tag="E")
                            nc.scalar.activation(E[:], Lps[(ci, g)][:], EXP)
                            Es[(ci, g)] = E

            # softmax normalization + P, per m-tile of the chunk
            Ps = {}
            for ci in range(CH):
                parts = []
                for g in range(NG):
                    t = Es[(ci, g)]
                    w = BG * NSH
                    while w > NSH:
                        w //= 2
                        nt = zp.tile([128, w], bf16, tag=f"zt{w}")
                        nc.gpsimd.tensor_tensor(nt[:], t[:, :w], t[:, w:2 * w], op=ADD)
                        t = nt
                    parts.append(t)
                while len(parts) > 1:
                    nxt = []
                    for i in range(0, len(parts), 2):
                        nt = zp.tile([128, NSH], bf16, tag="ztg")
                        nc.gpsimd.tensor_tensor(nt[:], parts[i][:], parts[i + 1][:],
                                                op=ADD)
                        nxt.append(nt)
                    parts = nxt
                zf = zp2.tile([128, NSH], f32, tag="zf")
                nc.vector.tensor_copy(zf[:], parts[0][:])
                rf = zp2.tile([128, NSH], f32, tag="rf")
                nc.vector.reciprocal(rf[:], zf[:])
                rb = zp2.tile([128, NSH], bf16, tag="rb")
                nc.vector.tensor_copy(rb[:], rf[:])
                rrep = zp2.tile([128, BG * NSH], bf16, tag="rrep")
                for j in range(BG):
                    nc.gpsimd.tensor_copy(rrep[:, j * NSH:(j + 1) * NSH], rb[:])
                for g in range(NG):
                    P = pp.tile([128, BG * NSH], f16, tag="P")
                    nc.gpsimd.tensor_tensor(P[:], Es[(ci, g)][:], rrep[:], op=MUL)
                    Ps[(ci, g)] = P

            # P @ [x|1]: chunk-accumulate in psum, then add into Y slab
            for b in range(B):
                g, j = b // BG, b % BG
                psy = ps_m.tile([NSH, YW], f32, tag="mm")
                for ci in range(CH):
                    nc.tensor.matmul(psy[:], Ps[(ci, g)][:, j * NSH:(j + 1) * NSH],
                                     slabs[ci][:, b, :],
                                     start=(ci == 0), stop=(ci == CH - 1))
                ysl = Y[:, b * YW:(b + 1) * YW]
                if ch == 0:
                    nc.vector.tensor_copy(ysl, psy[:])
                else:
                    nc.vector.tensor_tensor(ysl, ysl, psy[:], op=ADD)

        # ---- finale: out[b] = Y[:, :O] @ Wv^T + rowsum * bv
        for b in range(B):
            ysl = Y[:, b * YW:(b + 1) * YW]
            yh = outp.tile([NSH, YW], f16, tag="yh")
            nc.vector.tensor_copy(yh[:], ysl)
            pst = ps_t.tile([128, OT * NSH], f16, tag="tp")
            for et in range(OT):
                nc.tensor.transpose(pst[:, et * NSH:(et + 1) * NSH],
                                    yh[:, et * 128:(et + 1) * 128], ident[:NSH, :NSH])
            psr = ps_t.tile([1, NSH], f16, tag="tp")
            nc.tensor.transpose(psr[:], yh[:, O:O + 1], ident[:NSH, :NSH])
            yT = outp.tile([128, OT * NSH], f16, tag="yT")
            nc.scalar.activation(yT[:], pst[:], COPY)
            rT = outp.tile([1, NSH], f16, tag="rT")
            nc.scalar.activation(rT[:], psr[:], COPY)
            pso = ps_m.tile([NSH, O], f32, tag="mm")
            for et in range(OT):
                nc.tensor.matmul(pso[:], yT[:, et * NSH:(et + 1) * NSH], wvT[et][:],
                                 start=(et == 0), stop=False)
            nc.tensor.matmul(pso[:], rT[:], bv_r[:], start=False, stop=True)
            ot_ = outp.tile([NSH, O], f32, tag="ot")
            nc.vector.tensor_copy(ot_[:], pso[:])
            nc.sync.dma_start(out.ap()[b], ot_[:])

    return nc


_CACHE = {}


def _compiled(B, N, D, O, NSH):
    key = (B, N, D, O, NSH)
    if key not in _CACHE:
        nc = bacc.Bacc(trn_type="TRN2", target_bir_lowering=False, debug=False,
                       num_devices=N_CORES)
        build_graph(nc, B, N, D, O, NSH)
        nc.compile()
        _CACHE[key] = nc
    return _CACHE[key]


def kernel(x, Wq, bq, Wk, bk, Wv, bv):
    x = np.asarray(x, dtype=np.float32)
    B, N, D = x.shape
    O = Wq.shape[0]
    NSH = N // N_CORES
    nc = _compiled(B, N, D, O, NSH)
    from concourse.bass_utils import run_bass_kernel_spmd
    ins = []
    for c in range(N_CORES):
        ins.append({
            "xf": x,
            "xs": np.ascontiguousarray(x[:, c * NSH:(c + 1) * NSH, :]),
            "wq": np.asarray(Wq, np.float32), "wk": np.asarray(Wk, np.float32),
            "wv": np.asarray(Wv, np.float32),
            "bq": np.asarray(bq, np.float32), "bk": np.asarray(bk, np.float32),
            "bv": np.asarray(bv, np.float32),
        })
    res = run_bass_kernel_spmd(nc, ins, core_ids=list(range(N_CORES)))
    return np.concatenate([res.results[c]["out"] for c in range(N_CORES)], axis=1)


# revision 9
# speedup vs baseline: 1.3355x; 1.3355x over previous
"""Batch-softmax attention kernel for 8 trn2 NeuronCores.

Math (reference):
  q = x @ Wq^T + bq ; k = x @ Wk^T + bk ; v = x @ Wv^T + bv     (B,N,256)
  logits[b,n,m] = q[b,n,:] . k[b,m,:]
  attn = softmax over the BATCH axis b (32 entries per (n,m) site)
  out = attn @ v

Distribution: data-parallel over N (sequence). Core c owns n-shard
[c*NSH,(c+1)*NSH). Each core recomputes k for all m locally (no
collectives), streams x over m-tiles, and uses
  out = (P @ [x|1]) @ [Wv^T;bv]        (P = attn)
to skip the v projection entirely (the ones column yields rowsum(P),
which carries the bv term through the second matmul).

Layouts: attention tiles are (m-partition, n-free) i.e. logits^T, so the
batch softmax is elementwise across per-b tiles and P^T directly feeds
P@x as the stationary operand. exp needs no max-subtraction: |logits| is
~<=40, safe in fp32. E is bf16 (range), matmul operands fp16, all
accumulation fp32. Biases fold into the projections via an augmented
contraction row (bias_row x ones_row).
"""

import numpy as np
from contextlib import ExitStack

import concourse.bass as bass
import concourse.tile as tile
from concourse import mybir, bacc

f32 = mybir.dt.float32
f16 = mybir.dt.float16
bf16 = mybir.dt.bfloat16
i32 = mybir.dt.int32
ADD = mybir.AluOpType.add
MUL = mybir.AluOpType.mult
EQ = mybir.AluOpType.is_equal
EXP = mybir.ActivationFunctionType.Exp
COPY = mybir.ActivationFunctionType.Copy

N_CORES = 8


def build_graph(nc, B, N, D, O, NSH, CH=2, BG=4, BQ=4):
    MT = N // 128          # m-tiles
    NCH = MT // CH         # chunks
    NG = B // BG           # logits-psum b-groups
    DT, OT = D // 128, O // 128
    M = CH * 128           # m positions per chunk

    xf = nc.dram_tensor("xf", [B, N, D], f32, kind="ExternalInput")
    xs = nc.dram_tensor("xs", [B, NSH, D], f32, kind="ExternalInput")
    wq = nc.dram_tensor("wq", [O, D], f32, kind="ExternalInput")
    wk = nc.dram_tensor("wk", [O, D], f32, kind="ExternalInput")
    wv = nc.dram_tensor("wv", [O, D], f32, kind="ExternalInput")
    bq = nc.dram_tensor("bq", [O], f32, kind="ExternalInput")
    bk = nc.dram_tensor("bk", [O], f32, kind="ExternalInput")
    bv = nc.dram_tensor("bv", [O], f32, kind="ExternalInput")
    out = nc.dram_tensor("out", [B, NSH, O], f32, kind="ExternalOutput")

    with ExitStack() as ctx:
        tc = ctx.enter_context(tile.TileContext(nc))
        const = ctx.enter_context(tc.tile_pool(name="const", bufs=1))
        wtmp = ctx.enter_context(tc.tile_pool(name="wtmp", bufs=2))
        xfp = ctx.enter_context(tc.tile_pool(name="xfp", bufs=3))
        xtp = ctx.enter_context(tc.tile_pool(name="xtp", bufs=2))
        ktp = ctx.enter_context(tc.tile_pool(name="ktp", bufs=4))
        ep = ctx.enter_context(tc.tile_pool(name="ep", bufs=CH * NG + 4))
        pp = ctx.enter_context(tc.tile_pool(name="pp", bufs=CH * NG + 2))
        zp = ctx.enter_context(tc.tile_pool(name="zp", bufs=10))
        zp2 = ctx.enter_context(tc.tile_pool(name="zp2", bufs=3))
        outp = ctx.enter_context(tc.tile_pool(name="outp", bufs=3))
        ps_l = ctx.enter_context(tc.tile_pool(name="ps_l", bufs=3, space="PSUM"))
        ps_t = ctx.enter_context(tc.tile_pool(name="ps_t", bufs=3, space="PSUM"))
        ps_m = ctx.enter_context(tc.tile_pool(name="ps_m", bufs=2, space="PSUM"))

        # ---- constants
        ia = const.tile([128, 128], i32)
        ib = const.tile([128, 128], i32)
        nc.gpsimd.iota(ia[:], pattern=[[1, 128]], channel_multiplier=0)
        nc.gpsimd.iota(ib[:], pattern=[[0, 128]], channel_multiplier=1)
        ident = const.tile([128, 128], f16)
        nc.vector.tensor_tensor(ident[:], ia[:], ib[:], op=EQ)
        ones_row = const.tile([1, 512], f16)
        nc.vector.memset(ones_row[:], 1.0)

        # ---- weights: W^T (d-part, o-free) f16; bias rows (1,O) f16
        def load_wT(wdram, name):
            wT = [const.tile([128, O], f16, name=f"{name}T{d}", tag=f"{name}T{d}") for d in range(DT)]
            for ot in range(OT):
                wfull = wtmp.tile([128, D], f32, tag="wf")
                nc.sync.dma_start(wfull[:], wdram.ap()[ot * 128:(ot + 1) * 128, :])
                wh = wtmp.tile([128, D], f16, tag="wh")
                nc.vector.tensor_copy(wh[:], wfull[:])
                for d in range(DT):
                    ps = ps_m.tile([128, 128], f16, tag="mm")
                    nc.tensor.transpose(ps[:], wh[:, d * 128:(d + 1) * 128], ident[:])
                    nc.scalar.activation(wT[d][:, ot * 128:(ot + 1) * 128], ps[:], COPY)
            return wT

        def load_brow(bdram, name):
            bfl = wtmp.tile([1, O], f32, tag="bf")
            nc.sync.dma_start(bfl[:], bdram.ap()[:])
            bh = const.tile([1, O], f16, tag=name)
            nc.vector.tensor_copy(bh[:], bfl[:])
            return bh

        wqT = load_wT(wq, "wq")
        wkT = load_wT(wk, "wk")
        wvT = load_wT(wv, "wv")
        bv_r = load_brow(bv, "bv")

        def load_bcol(bdram, name):
            cols = []
            for ot in range(OT):
                c = const.tile([128, 1], f32, name=f"{name}c{ot}", tag=f"{name}c{ot}")
                nc.sync.dma_start(c[:], bdram.ap()[ot * 128:(ot + 1) * 128])
                cols.append(c)
            return cols
        bq_c = load_bcol(bq, "bq")
        bk_c = load_bcol(bk, "bk")
        IDENT_ACT = mybir.ActivationFunctionType.Identity

        # ---- setup: q^T (o-part, n-free) per b. slab slice (b,ot) at
        # [(b*OT+ot)*NSH].
        qT = const.tile([128, B * OT * NSH], f16)
        xss = const.tile([NSH, B * D], f16, tag="xss")
        xs3 = xss[:].rearrange("p (b d) -> p b d", b=B)
        nc.gpsimd.dma_start(xs3[:, :, :], xs.ap()[:, :, :].rearrange("b n d -> n b d"))
        for b in range(B):
            pst = ps_t.tile([128, DT * NSH], f16, tag="tp")
            for d in range(DT):
                nc.tensor.transpose(pst[:, d * NSH:(d + 1) * NSH],
                                    xs3[:, b, d * 128:(d + 1) * 128],
                                    ident[:NSH, :NSH])
            xsT = wtmp.tile([128, DT * NSH], f16, tag="xsT")
            nc.vector.tensor_copy(xsT[:], pst[:])
            psq = ps_m.tile([128, OT * NSH], f32, tag="mm")
            for ot in range(OT):
                sl = psq[:, ot * NSH:(ot + 1) * NSH]
                for d in range(DT):
                    nc.tensor.matmul(sl, wqT[d][:, ot * 128:(ot + 1) * 128],
                                     xsT[:, d * NSH:(d + 1) * NSH],
                                     start=(d == 0), stop=(d == DT - 1))
                nc.scalar.activation(
                    qT[:, (b * OT + ot) * NSH:(b * OT + ot + 1) * NSH], sl,
                    IDENT_ACT, bias=bq_c[ot][:])

        # ---- Y accumulator (n-part, b x (O+1)) f32
        YW = O + 1
        Y = const.tile([NSH, B * YW], f32)

        # ---- stream over chunks of CH m-tiles
        for ch in range(NCH):
            slabs = []
            for ci in range(CH):
                mt = ch * CH + ci
                slab = xfp.tile([128, B * (D + 1)], f16, tag="xslab")
                s3 = slab[:].rearrange("p (b d) -> p b d", b=B)
                nc.vector.memset(s3[:, :, D:D + 1], 1.0)
                nc.gpsimd.dma_start(
                    s3[:, :, 0:D],
                    xf.ap()[:, mt * 128:(mt + 1) * 128, :].rearrange("b n d -> n b d"))
                slabs.append(s3)

            Lps, Es, Zacc = {}, {}, {}
            for b0 in range(0, B, BQ):
                tps = [ps_t.tile([128, BQ * M], f16, name=f"tps{d}", tag="tp") for d in range(DT)]
                xT = [xtp.tile([128, BQ * M], f16, name=f"xT{d}", tag=f"xT{d}") for d in range(DT)]
                for bb in range(BQ):
                    for ci in range(CH):
                        for d in range(DT):
                            nc.tensor.transpose(
                                tps[d][:, (bb * CH + ci) * 128:(bb * CH + ci + 1) * 128],
                                slabs[ci][:, b0 + bb, d * 128:(d + 1) * 128], ident[:])
                for d in range(DT):
                    nc.vector.tensor_copy(xT[d][:], tps[d][:])
                for bb in range(BQ):
                    b = b0 + bb
                    psk = ps_m.tile([128, OT * M], f32, tag="mm")
                    kT = ktp.tile([128, OT * M], f16, tag="kT")
                    for ot in range(OT):
                        sl = psk[:, ot * M:(ot + 1) * M]
                        for d in range(DT):
                            nc.tensor.matmul(sl, wkT[d][:, ot * 128:(ot + 1) * 128],
                                             xT[d][:, bb * M:(bb + 1) * M],
                                             start=(d == 0), stop=(d == DT - 1))
                        nc.scalar.activation(kT[:, ot * M:(ot + 1) * M], sl,
                                             IDENT_ACT, bias=bk_c[ot][:])

                    g, j = b // BG, b % BG
                    for ci in range(CH):
                        if j == 0:
                            Lps[(ci, g)] = ps_l.tile([128, BG * NSH], f32, name="lps", tag="lps")
                        sl = Lps[(ci, g)][:, j * NSH:(j + 1) * NSH]
                        for ot in range(OT):
                            nc.tensor.matmul(
                                sl, kT[:, ot * M + ci * 128:ot * M + (ci + 1) * 128],
                                qT[:, (b * OT + ot) * NSH:(b * OT + ot + 1) * NSH],
                                start=(ot == 0), stop=(ot == OT - 1))
                        if j == BG - 1:
                            E = ep.tile([128, BG * NSH], bf16, tag="E")
                            nc.scalar.activation(E[:], Lps[(ci, g)][:], EXP)
                            Es[(ci, g)] = E
                            # incremental Z: reduce this group now (GpSimd)
                            t = E
                            w = BG * NSH
                            while w > NSH:
                                w //= 2
                                nt = zp.tile([128, w], bf16, name=f"nt{w}", tag=f"zt{w}")
                                nc.gpsimd.tensor_tensor(nt[:], t[:, :w], t[:, w:2 * w], op=ADD)
                                t = nt
                            if g == 0:
                                Zacc[ci] = t
                            else:
                                za = zp.tile([128, NSH], bf16, name="za", tag="ztg")
                                nc.gpsimd.tensor_tensor(za[:], Zacc[ci][:], t[:], op=ADD)
                                Zacc[ci] = za

            # softmax normalization + P (DVE), per m-tile of the chunk
            Ps = {}
            for ci in range(CH):
                zf = zp2.tile([128, NSH], f32, tag="zf")
                nc.vector.tensor_copy(zf[:], Zacc[ci][:])
                rf = zp2.tile([128, NSH], f32, tag="rf")
                nc.vector.reciprocal(rf[:], zf[:])
                rb = zp2.tile([128, NSH], bf16, tag="rb")
                nc.vector.tensor_copy(rb[:], rf[:])
                rrep = zp2.tile([128, BG * NSH], bf16, tag="rrep")
                for j in range(BG):
                    nc.gpsimd.tensor_copy(rrep[:, j * NSH:(j + 1) * NSH], rb[:])
                for g in range(NG):
                    P = pp.tile([128, BG * NSH], f16, tag="P")
                    nc.vector.tensor_tensor(P[:], Es[(ci, g)][:], rrep[:], op=MUL)
                    Ps[(ci, g)] = P

            # P @ [x|1]: chunk-accumulate in psum, then add into Y slab
            for b in range(B):
                g, j = b // BG, b % BG
                psy = ps_m.tile([NSH, YW], f32, tag="mm")
                for ci in range(CH):
                    nc.tensor.matmul(psy[:], Ps[(ci, g)][:, j * NSH:(j + 1) * NSH],
                                     slabs[ci][:, b, :],
                                     start=(ci == 0), stop=(ci == CH - 1))
                ysl = Y[:, b * YW:(b + 1) * YW]
                if ch == 0:
                    nc.vector.tensor_copy(ysl, psy[:])
                else:
                    nc.vector.tensor_tensor(ysl, ysl, psy[:], op=ADD)

        # ---- finale: out[b] = Y[:, :O] @ Wv^T + rowsum * bv
        for b in range(B):
            ysl = Y[:, b * YW:(b + 1) * YW]
            yh = outp.tile([NSH, YW], f16, tag="yh")
            nc.vector.tensor_copy(yh[:], ysl)
            pst = ps_t.tile([128, OT * NSH], f16, tag="tp")
            for et in range(OT):
                nc.tensor.transpose(pst[:, et * NSH:(et + 1) * NSH],
                                    yh[:, et * 128:(et + 1) * 128], ident[:NSH, :NSH])
            psr = ps_t.tile([1, NSH], f16, tag="tp")
            nc.tensor.transpose(psr[:], yh[:, O:O + 1], ident[:NSH, :NSH])
            yT = outp.tile([128, OT * NSH], f16, tag="yT")
            nc.scalar.activation(yT[:], pst[:], COPY)
            rT = outp.tile([1, NSH], f16, tag="rT")
            nc.scalar.activation(rT[:], psr[:], COPY)
            pso = ps_m.tile([NSH, O], f32, tag="mm")
            for et in range(OT):
                nc.tensor.matmul(pso[:], yT[:, et * NSH:(et + 1) * NSH], wvT[et][:],
                                 start=(et == 0), stop=False)
            nc.tensor.matmul(pso[:], rT[:], bv_r[:], start=False, stop=True)
            ot_ = outp.tile([NSH, O], f32, tag="ot")
            nc.vector.tensor_copy(ot_[:], pso[:])
            nc.sync.dma_start(out.ap()[b], ot_[:])

    return nc


_CACHE = {}


def _compiled(B, N, D, O, NSH):
    key = (B, N, D, O, NSH)
    if key not in _CACHE:
        nc = bacc.Bacc(trn_type="TRN2", target_bir_lowering=False, debug=False,
                       num_devices=N_CORES)
        build_graph(nc, B, N, D, O, NSH)
        nc.compile()
        _CACHE[key] = nc
    return _CACHE[key]


def kernel(x, Wq, bq, Wk, bk, Wv, bv):
    x = np.asarray(x, dtype=np.float32)
    B, N, D = x.shape
    O = Wq.shape[0]
    NSH = N // N_CORES
    nc = _compiled(B, N, D, O, NSH)
    from concourse.bass_utils import run_bass_kernel_spmd
    ins = []
    for c in range(N_CORES):
        ins.append({
            "xf": x,
            "xs": np.ascontiguousarray(x[:, c * NSH:(c + 1) * NSH, :]),
            "wq": np.asarray(Wq, np.float32), "wk": np.asarray(Wk, np.float32),
            "wv": np.asarray(Wv, np.float32),
            "bq": np.asarray(bq, np.float32), "bk": np.asarray(bk, np.float32),
            "bv": np.asarray(bv, np.float32),
        })
    res = run_bass_kernel_spmd(nc, ins, core_ids=list(range(N_CORES)))
    return np.concatenate([res.results[c]["out"] for c in range(N_CORES)], axis=1)


# revision 11
# speedup vs baseline: 1.4940x; 1.1187x over previous
"""Batch-softmax attention kernel for 8 trn2 NeuronCores.

Math (reference):
  q = x @ Wq^T + bq ; k = x @ Wk^T + bk ; v = x @ Wv^T + bv     (B,N,256)
  logits[b,n,m] = q[b,n,:] . k[b,m,:]
  attn = softmax over the BATCH axis b (32 entries per (n,m) site)
  out = attn @ v

Distribution: data-parallel over N (sequence). Core c owns n-shard
[c*NSH,(c+1)*NSH). Each core recomputes k for all m locally (no
collectives), streams x over m-tiles, and uses
  out = (P @ [x|1]) @ [Wv^T;bv]        (P = attn)
to skip the v projection entirely (the ones column yields rowsum(P),
which carries the bv term through the second matmul).

Layouts: attention tiles are (m-partition, n-free) i.e. logits^T, so the
batch softmax is elementwise across per-b tiles and P^T directly feeds
P@x as the stationary operand. exp needs no max-subtraction: |logits| is
~<=40, safe in fp32. E is bf16 (range), matmul operands fp16, all
accumulation fp32. Biases fold into the projections via an augmented
contraction row (bias_row x ones_row).
"""

import numpy as np
from contextlib import ExitStack

import concourse.bass as bass
import concourse.tile as tile
from concourse import mybir, bacc

f32 = mybir.dt.float32
f16 = mybir.dt.float16
bf16 = mybir.dt.bfloat16
i32 = mybir.dt.int32
ADD = mybir.AluOpType.add
MUL = mybir.AluOpType.mult
EQ = mybir.AluOpType.is_equal
EXP = mybir.ActivationFunctionType.Exp
COPY = mybir.ActivationFunctionType.Copy

N_CORES = 8


def build_graph(nc, B, N, D, O, NSH, CH=2, BG=4, BQ=4):
    MT = N // 128          # m-tiles
    NCH = MT // CH         # chunks
    NG = B // BG           # logits-psum b-groups
    DT, OT = D // 128, O // 128
    M = CH * 128           # m positions per chunk

    xf = nc.dram_tensor("xf", [B, N, D], f32, kind="ExternalInput")
    xs = nc.dram_tensor("xs", [B, NSH, D], f32, kind="ExternalInput")
    wq = nc.dram_tensor("wq", [O, D], f32, kind="ExternalInput")
    wk = nc.dram_tensor("wk", [O, D], f32, kind="ExternalInput")
    wv = nc.dram_tensor("wv", [O, D], f32, kind="ExternalInput")
    bq = nc.dram_tensor("bq", [O], f32, kind="ExternalInput")
    bk = nc.dram_tensor("bk", [O], f32, kind="ExternalInput")
    bv = nc.dram_tensor("bv", [O], f32, kind="ExternalInput")
    out = nc.dram_tensor("out", [B, NSH, O], f32, kind="ExternalOutput")

    with ExitStack() as ctx:
        tc = ctx.enter_context(tile.TileContext(nc))
        const = ctx.enter_context(tc.tile_pool(name="const", bufs=1))
        wtmp = ctx.enter_context(tc.tile_pool(name="wtmp", bufs=2))
        xfp = ctx.enter_context(tc.tile_pool(name="xfp", bufs=4))
        xtp = ctx.enter_context(tc.tile_pool(name="xtp", bufs=2))
        ktp = ctx.enter_context(tc.tile_pool(name="ktp", bufs=3))
        ep = ctx.enter_context(tc.tile_pool(name="ep", bufs=CH * NG + 2))
        pp = ctx.enter_context(tc.tile_pool(name="pp", bufs=CH * NG + 1))
        zp = ctx.enter_context(tc.tile_pool(name="zp", bufs=6))
        zp2 = ctx.enter_context(tc.tile_pool(name="zp2", bufs=3))
        outp = ctx.enter_context(tc.tile_pool(name="outp", bufs=3))
        ps_l = ctx.enter_context(tc.tile_pool(name="ps_l", bufs=3, space="PSUM"))
        ps_t = ctx.enter_context(tc.tile_pool(name="ps_t", bufs=3, space="PSUM"))
        ps_m = ctx.enter_context(tc.tile_pool(name="ps_m", bufs=2, space="PSUM"))

        # ---- constants
        ia = const.tile([128, 128], i32)
        ib = const.tile([128, 128], i32)
        nc.gpsimd.iota(ia[:], pattern=[[1, 128]], channel_multiplier=0)
        nc.gpsimd.iota(ib[:], pattern=[[0, 128]], channel_multiplier=1)
        ident = const.tile([128, 128], f16)
        nc.vector.tensor_tensor(ident[:], ia[:], ib[:], op=EQ)
        ones_row = const.tile([1, 512], f16)
        nc.vector.memset(ones_row[:], 1.0)

        # ---- weights: W^T (d-part, o-free) f16; bias rows (1,O) f16
        def load_wT(wdram, name):
            wT = [const.tile([128, O], f16, name=f"{name}T{d}", tag=f"{name}T{d}") for d in range(DT)]
            for ot in range(OT):
                wfull = wtmp.tile([128, D], f32, tag="wf")
                nc.sync.dma_start(wfull[:], wdram.ap()[ot * 128:(ot + 1) * 128, :])
                wh = wtmp.tile([128, D], f16, tag="wh")
                nc.vector.tensor_copy(wh[:], wfull[:])
                for d in range(DT):
                    ps = ps_m.tile([128, 128], f16, tag="mm")
                    nc.tensor.transpose(ps[:], wh[:, d * 128:(d + 1) * 128], ident[:])
                    nc.scalar.activation(wT[d][:, ot * 128:(ot + 1) * 128], ps[:], COPY)
            return wT

        def load_brow(bdram, name):
            bfl = wtmp.tile([1, O], f32, tag="bf")
            nc.sync.dma_start(bfl[:], bdram.ap()[:])
            bh = const.tile([1, O], f16, tag=name)
            nc.vector.tensor_copy(bh[:], bfl[:])
            return bh

        wqT = load_wT(wq, "wq")
        wkT = load_wT(wk, "wk")
        wvT = load_wT(wv, "wv")
        bv_r = load_brow(bv, "bv")

        def load_bcol(bdram, name):
            cols = []
            for ot in range(OT):
                c = const.tile([128, 1], f32, name=f"{name}c{ot}", tag=f"{name}c{ot}")
                nc.sync.dma_start(c[:], bdram.ap()[ot * 128:(ot + 1) * 128])
                cols.append(c)
            return cols
        bq_c = load_bcol(bq, "bq")
        bk_c = load_bcol(bk, "bk")
        IDENT_ACT = mybir.ActivationFunctionType.Identity

        # ---- setup: q^T (o-part, n-free) per b. slab slice (b,ot) at
        # [(b*OT+ot)*NSH].
        qT = const.tile([128, B * OT * NSH], f16)
        xss = const.tile([NSH, B * D], f16, tag="xss")
        xs3 = xss[:].rearrange("p (b d) -> p b d", b=B)
        nc.gpsimd.dma_start(xs3[:, :, :], xs.ap()[:, :, :].rearrange("b n d -> n b d"))
        for b in range(B):
            pst = ps_t.tile([128, DT * NSH], f16, tag="tp")
            for d in range(DT):
                nc.tensor.transpose(pst[:, d * NSH:(d + 1) * NSH],
                                    xs3[:, b, d * 128:(d + 1) * 128],
                                    ident[:NSH, :NSH])
            xsT = wtmp.tile([128, DT * NSH], f16, tag="xsT")
            nc.vector.tensor_copy(xsT[:], pst[:])
            psq = ps_m.tile([128, OT * NSH], f32, tag="mm")
            for ot in range(OT):
                sl = psq[:, ot * NSH:(ot + 1) * NSH]
                for d in range(DT):
                    nc.tensor.matmul(sl, wqT[d][:, ot * 128:(ot + 1) * 128],
                                     xsT[:, d * NSH:(d + 1) * NSH],
                                     start=(d == 0), stop=(d == DT - 1))
                nc.scalar.activation(
                    qT[:, (b * OT + ot) * NSH:(b * OT + ot + 1) * NSH], sl,
                    IDENT_ACT, bias=bq_c[ot][:])

        # ---- Y accumulator (n-part, b x (O+1)) f32
        YW = O + 1
        Y = const.tile([NSH, B * YW], f32)

        # ---- stream over chunks of CH m-tiles (P@x of chunk c is emitted
        # after the first b-quad of chunk c+1 so the PE never stalls on the
        # softmax chain at chunk boundaries)
        def emit_pax(ch, slabs, Ps):
            for b in range(B):
                g, j = b // BG, b % BG
                psy = ps_m.tile([NSH, YW], f32, name="psy", tag="mm")
                for ci in range(CH):
                    nc.tensor.matmul(psy[:], Ps[(ci, g)][:, j * NSH:(j + 1) * NSH],
                                     slabs[ci][:, b, :],
                                     start=(ci == 0), stop=(ci == CH - 1))
                ysl = Y[:, b * YW:(b + 1) * YW]
                if ch == 0:
                    nc.vector.tensor_copy(ysl, psy[:])
                else:
                    nc.vector.tensor_tensor(ysl, ysl, psy[:], op=ADD)

        pending = None
        for ch in range(NCH):
            slabs = []
            for ci in range(CH):
                mt = ch * CH + ci
                slab = xfp.tile([128, B * (D + 1)], f16, tag="xslab")
                s3 = slab[:].rearrange("p (b d) -> p b d", b=B)
                nc.vector.memset(s3[:, :, D:D + 1], 1.0)
                nc.gpsimd.dma_start(
                    s3[:, :, 0:D],
                    xf.ap()[:, mt * 128:(mt + 1) * 128, :].rearrange("b n d -> n b d"))
                slabs.append(s3)

            Lps, Es, Zacc = {}, {}, {}
            for iq, b0 in enumerate(range(0, B, BQ)):
                tps = [ps_t.tile([128, BQ * M], f16, name=f"tps{d}", tag="tp") for d in range(DT)]
                xT = [xtp.tile([128, BQ * M], f16, name=f"xT{d}", tag=f"xT{d}") for d in range(DT)]
                for bb in range(BQ):
                    for ci in range(CH):
                        for d in range(DT):
                            nc.tensor.transpose(
                                tps[d][:, (bb * CH + ci) * 128:(bb * CH + ci + 1) * 128],
                                slabs[ci][:, b0 + bb, d * 128:(d + 1) * 128], ident[:])
                for d in range(DT):
                    nc.vector.tensor_copy(xT[d][:], tps[d][:])
                for bb in range(BQ):
                    b = b0 + bb
                    psk = ps_m.tile([128, OT * M], f32, tag="mm")
                    kT = ktp.tile([128, OT * M], f16, tag="kT")
                    for ot in range(OT):
                        sl = psk[:, ot * M:(ot + 1) * M]
                        for d in range(DT):
                            nc.tensor.matmul(sl, wkT[d][:, ot * 128:(ot + 1) * 128],
                                             xT[d][:, bb * M:(bb + 1) * M],
                                             start=(d == 0), stop=(d == DT - 1))
                        nc.scalar.activation(kT[:, ot * M:(ot + 1) * M], sl,
                                             IDENT_ACT, bias=bk_c[ot][:])

                    g, j = b // BG, b % BG
                    for ci in range(CH):
                        if j == 0:
                            Lps[(ci, g)] = ps_l.tile([128, BG * NSH], f32, name="lps", tag="lps")
                        sl = Lps[(ci, g)][:, j * NSH:(j + 1) * NSH]
                        for ot in range(OT):
                            nc.tensor.matmul(
                                sl, kT[:, ot * M + ci * 128:ot * M + (ci + 1) * 128],
                                qT[:, (b * OT + ot) * NSH:(b * OT + ot + 1) * NSH],
                                start=(ot == 0), stop=(ot == OT - 1))
                        if j == BG - 1:
                            E = ep.tile([128, BG * NSH], bf16, tag="E")
                            nc.scalar.activation(E[:], Lps[(ci, g)][:], EXP)
                            Es[(ci, g)] = E
                            # incremental Z: reduce this group now (GpSimd)
                            t = E
                            w = BG * NSH
                            while w > NSH:
                                w //= 2
                                nt = zp.tile([128, w], bf16, name=f"nt{w}", tag=f"zt{w}")
                                nc.gpsimd.tensor_tensor(nt[:], t[:, :w], t[:, w:2 * w], op=ADD)
                                t = nt
                            if g == 0:
                                Zacc[ci] = t
                            else:
                                za = zp.tile([128, NSH], bf16, name="za", tag="ztg")
                                nc.gpsimd.tensor_tensor(za[:], Zacc[ci][:], t[:], op=ADD)
                                Zacc[ci] = za
                if iq == 0 and pending is not None:
                    emit_pax(*pending)
                    pending = None

            # softmax normalization + P (DVE), per m-tile of the chunk
            Ps = {}
            for ci in range(CH):
                zf = zp2.tile([128, NSH], f32, tag="zf")
                nc.vector.tensor_copy(zf[:], Zacc[ci][:])
                rf = zp2.tile([128, NSH], f32, tag="rf")
                nc.vector.reciprocal(rf[:], zf[:])
                rb = zp2.tile([128, NSH], bf16, tag="rb")
                nc.vector.tensor_copy(rb[:], rf[:])
                rrep = zp2.tile([128, BG * NSH], bf16, tag="rrep")
                for j in range(BG):
                    nc.gpsimd.tensor_copy(rrep[:, j * NSH:(j + 1) * NSH], rb[:])
                for g in range(NG):
                    P = pp.tile([128, BG * NSH], f16, tag="P")
                    nc.vector.tensor_tensor(P[:], Es[(ci, g)][:], rrep[:], op=MUL)
                    Ps[(ci, g)] = P

            pending = (ch, slabs, Ps)


        if pending is not None:
            emit_pax(*pending)
            pending = None

        # ---- finale: out[b] = Y[:, :O] @ Wv^T + rowsum * bv
        for b in range(B):
            ysl = Y[:, b * YW:(b + 1) * YW]
            yh = outp.tile([NSH, YW], f16, tag="yh")
            nc.vector.tensor_copy(yh[:], ysl)
            pst = ps_t.tile([128, OT * NSH], f16, tag="tp")
            for et in range(OT):
                nc.tensor.transpose(pst[:, et * NSH:(et + 1) * NSH],
                                    yh[:, et * 128:(et + 1) * 128], ident[:NSH, :NSH])
            psr = ps_t.tile([1, NSH], f16, tag="tp")
            nc.tensor.transpose(psr[:], yh[:, O:O + 1], ident[:NSH, :NSH])
            yT = outp.tile([128, OT * NSH], f16, tag="yT")
            nc.scalar.activation(yT[:], pst[:], COPY)
            rT = outp.tile([1, NSH], f16, tag="rT")
            nc.scalar.activation(rT[:], psr[:], COPY)
            pso = ps_m.tile([NSH, O], f32, tag="mm")
            for et in range(OT):
                nc.tensor.matmul(pso[:], yT[:, et * NSH:(et + 1) * NSH], wvT[et][:],
                                 start=(et == 0), stop=False)
            nc.tensor.matmul(pso[:], rT[:], bv_r[:], start=False, stop=True)
            ot_ = outp.tile([NSH, O], f32, tag="ot")
            nc.vector.tensor_copy(ot_[:], pso[:])
            nc.sync.dma_start(out.ap()[b], ot_[:])

    return nc


_CACHE = {}


def _compiled(B, N, D, O, NSH):
    key = (B, N, D, O, NSH)
    if key not in _CACHE:
        nc = bacc.Bacc(trn_type="TRN2", target_bir_lowering=False, debug=False,
                       num_devices=N_CORES)
        build_graph(nc, B, N, D, O, NSH)
        nc.compile()
        _CACHE[key] = nc
    return _CACHE[key]


def kernel(x, Wq, bq, Wk, bk, Wv, bv):
    x = np.asarray(x, dtype=np.float32)
    B, N, D = x.shape
    O = Wq.shape[0]
    NSH = N // N_CORES
    nc = _compiled(B, N, D, O, NSH)
    from concourse.bass_utils import run_bass_kernel_spmd
    ins = []
    for c in range(N_CORES):
        ins.append({
            "xf": x,
            "xs": np.ascontiguousarray(x[:, c * NSH:(c + 1) * NSH, :]),
            "wq": np.asarray(Wq, np.float32), "wk": np.asarray(Wk, np.float32),
            "wv": np.asarray(Wv, np.float32),
            "bq": np.asarray(bq, np.float32), "bk": np.asarray(bk, np.float32),
            "bv": np.asarray(bv, np.float32),
        })
    res = run_bass_kernel_spmd(nc, ins, core_ids=list(range(N_CORES)))
    return np.concatenate([res.results[c]["out"] for c in range(N_CORES)], axis=1)
